# revision 24
# baseline (speedup 1.0000x reference)
"""MASA agent-attention kernel for Trainium2, 8-core SPMD. v2.

Sharding: core = (batch b in 0..3) x (head-group hg in 0..1).
Each core computes conv1x1 + depthwise3x3 for its 4 heads' q/k/v/a
channels (384 of 768), the agent attention for those heads, and SimAM
over its 96 output channels. No cross-core communication.

Slab layout (conv output channel packing, 3 slabs of 128):
  slab0 -> dw0 = [q 0:96 | a 0:32]
  slab1 -> dw1 = [k 0:96 | a 32:64]
  slab2 -> dw2 = [v 0:96 | a 64:96]

Key design points vs v1 baseline:
  - v is transposed per 128-pixel chunk on the PE (matmul transpose)
    instead of 384 serial DMA transposes (which cost ~800us).
  - agent_v accumulated as ONE matmul per chunk with stationary
    [vT | ones-col] so the k-softmax denominator comes out for free.
  - agent pooling via f16 tensor_tensor adder trees (2x DVE mode)
    staged in dead a-rows of the other dw buffers.
  - q/k packed contiguously -> fewer fixup DMAs; l2norm ops spread
    over ACT/DVE/GPSIMD.
  - output in f16 (cast to f32 host-side).

Engine-op partition windows must be 32-aligned and (base==0 or count<=32).
"""

import sys
import types
import numpy as np

import concourse.bacc as bacc
import concourse.bass as bass
import concourse.mybir as mybir
from concourse.tile import TileContext
from concourse.bass_utils import run_bass_kernel_spmd

F16 = mybir.dt.float16
F32 = mybir.dt.float32
AX = mybir.AxisListType
OP = mybir.AluOpType
AF = mybir.ActivationFunctionType

B, C, H, W = 4, 192, 128, 128
N = H * W              # 16384
M_AG = 64              # agent tokens per head
E_LAMBDA = 1e-4
RS = 130               # padded row stride for pre
PREFREE = RS * RS      # 16900

NB2 = 16               # 1024-col blocks
BLK2 = 1024
NGRP = 64              # k-side 2-chunk groups

TAPS = [(dy, dx) for dy in (-1, 0, 1) for dx in (-1, 0, 1)]
# tap split per slab: PE 7, DVE 1 (folds psum->sbuf), GPSIMD 1
PE_TAPS = {
    0: [t for t in TAPS if t[1] != 1],
    1: [t for t in TAPS if t[1] != 1] + [(-1, 1)],
    2: [t for t in TAPS if t[1] != 1] + [(-1, 1)],
}
DVE_TAPS = {
    0: [(-1, 1), (0, 1), (1, 1)],
    1: [(0, 1), (1, 1)],
    2: [(0, 1), (1, 1)],
}
GP_TAPS = {s: [] for s in range(3)}
WDIAG_SLOT = {}
for _s in range(3):
    for _t in PE_TAPS[_s]:
        WDIAG_SLOT[(_s, _t[0], _t[1])] = len(WDIAG_SLOT)
NDIAG = len(WDIAG_SLOT)

# pool-tree staging: strip s stages in a-rows (96:128) of dws[STAGE_OF[s]]
STAGE_OF = {0: 2, 1: 0, 2: 1}
# stage col offsets for the tree levels (units: f16 elements)
ST_L1, ST_L2, ST_L3, ST_L4 = 0, 8192, 12288, 14336
ST_M1, ST_M2, ST_M3, ST_M4 = 15360, 15872, 16128, 16256


def _install_ntff_hook():
    try:
        import antenv.axon_hooks  # noqa: F401
        return
    except ImportError:
        pass
    try:
        from trn_agent_boot.trn_boot import _ntff_profile_via_ctypes
        hook = _ntff_profile_via_ctypes('/opt/axon/libaxon_pjrt.so')
        mod = types.ModuleType("antenv.axon_hooks")
        mod.get_axon_ntff_profile_hook = lambda: hook
        mod.set_axon_ntff_profile_hook = lambda h: None
        sys.modules["antenv.axon_hooks"] = mod
    except Exception:
        pass


def build_nc(debug=False):
    nc = bacc.Bacc("TRN2", target_bir_lowering=False, debug=False, num_devices=8)

    # ---- DRAM I/O ----
    xin = nc.dram_tensor("xin", [192, N], F16, kind="ExternalInput").ap()
    w1a = nc.dram_tensor("w1a", [96, 384], F16, kind="ExternalInput").ap()
    w1b = nc.dram_tensor("w1b", [96, 384], F16, kind="ExternalInput").ap()
    wdiag = nc.dram_tensor("wdiag", [128, NDIAG * 128], F16, kind="ExternalInput").ap()
    wtap = nc.dram_tensor("wtap", [128, 27], F32, kind="ExternalInput").ap()
    tmp_rep_d = nc.dram_tensor("tmp_rep", [96, 1], F32, kind="ExternalInput").ap()
    pat = nc.dram_tensor("pat", [128, 384], F16, kind="ExternalInput").ap()
    out_d = nc.dram_tensor("out", [96, N], F16, kind="ExternalOutput").ap()
    if debug:
        dbg_pre = nc.dram_tensor("dbg_pre", [128, PREFREE], F16, kind="ExternalOutput").ap()
        dbg_dw0 = nc.dram_tensor("dbg_dw0", [128, N], F16, kind="ExternalOutput").ap()
        dbg_dw1 = nc.dram_tensor("dbg_dw1", [128, N], F16, kind="ExternalOutput").ap()
        dbg_dw2 = nc.dram_tensor("dbg_dw2", [128, N], F16, kind="ExternalOutput").ap()
        dbg_ag = nc.dram_tensor("dbg_ag", [96, 256], F16, kind="ExternalOutput").ap()
        dbg_qn = nc.dram_tensor("dbg_qn", [128, N], F16, kind="ExternalOutput").ap()
        dbg_kn = nc.dram_tensor("dbg_kn", [128, N], F16, kind="ExternalOutput").ap()
        dbg_agv = nc.dram_tensor("dbg_agv", [97, 256], F16, kind="ExternalOutput").ap()
        dbg_av0 = nc.dram_tensor("dbg_av0", [128, 48], F16, kind="ExternalOutput").ap()
        dbg_av1 = nc.dram_tensor("dbg_av1", [128, 48], F16, kind="ExternalOutput").ap()
        dbg_xa = nc.dram_tensor("dbg_xa", [96, N], F16, kind="ExternalOutput").ap()

    # ---- persistent SBUF ----
    scratch = nc.alloc_sbuf_tensor("scratch", [128, PREFREE], F16).ap()
    dw0 = nc.alloc_sbuf_tensor("dw0", [128, N], F16).ap()
    dw1 = nc.alloc_sbuf_tensor("dw1", [128, N], F16).ap()
    dw2 = nc.alloc_sbuf_tensor("dw2", [128, N], F16).ap()
    dws = [dw0, dw1, dw2]
    w1a_s = nc.alloc_sbuf_tensor("w1a_s", [96, 384], F16).ap()
    w1b_s = nc.alloc_sbuf_tensor("w1b_s", [96, 384], F16).ap()
    wdiag_s = nc.alloc_sbuf_tensor("wdiag_s", [128, NDIAG * 128], F16).ap()
    wtap_s = nc.alloc_sbuf_tensor("wtap_s", [128, 27], F32).ap()
    ident = nc.alloc_sbuf_tensor("ident", [128, 128], F16).ap()
    ones_q = nc.alloc_sbuf_tensor("ones_q", [96, 128], F16).ap()
    dv_ones = nc.alloc_sbuf_tensor("dv_ones", [128, 128], F16).ap()
    ag_full = nc.alloc_sbuf_tensor("ag_full", [96, 256], F16).ap()
    agf = nc.alloc_sbuf_tensor("agf", [96, M_AG], F16).ap()
    agfs = nc.alloc_sbuf_tensor("agfs", [96, M_AG], F16).ap()
    tmp_rep = nc.alloc_sbuf_tensor("tmp_rep_s", [96, 1], F32).ap()
    vts = nc.alloc_sbuf_tensor("vts", [128, 512], F16).ap()
    agv_sb = nc.alloc_sbuf_tensor("agv_sb", [97, 256], F16).ap()
    av_l0 = nc.alloc_sbuf_tensor("av_l0", [128, 128], F16).ap()
    av_l1 = nc.alloc_sbuf_tensor("av_l1", [128, 128], F16).ap()
    rden = nc.alloc_sbuf_tensor("rden", [128, 2], F32).ap()
    mu_parts = nc.alloc_sbuf_tensor("mu_parts", [48, 2 * NB2], F32).ap()
    x2_parts = nc.alloc_sbuf_tensor("x2_parts", [48, 2 * NB2], F32).ap()
    mu_neg = nc.alloc_sbuf_tensor("mu_neg", [96, 1], F32).ap()
    mub = nc.alloc_sbuf_tensor("mub", [48, 2], F32).ap()
    x2b = nc.alloc_sbuf_tensor("x2b", [48, 2], F32).ap()
    x2s = nc.alloc_sbuf_tensor("x2s", [96, 1], F32).ap()
    mu2 = nc.alloc_sbuf_tensor("mu2", [96, 1], F32).ap()
    sden = nc.alloc_sbuf_tensor("sden", [96, 1], F32).ap()
    s_ch = nc.alloc_sbuf_tensor("s_ch", [96, 1], F32).ap()
    rsq = nc.alloc_sbuf_tensor("rsq", [96, 1], F32).ap()
    bsg = nc.alloc_sbuf_tensor("bsg", [96, 1], F32).ap()
    half_s = nc.alloc_sbuf_tensor("half_s", [96, 1], F32).ap()

    # aliases (sequential reuse of big buffers)
    pre3 = scratch.rearrange("p (y x) -> p y x", x=RS)   # padded conv out
    sig = scratch[:, 0:N]                                # simam sigmoid (rows 0:96)
    x_attn = dw1[0:96, :]                                # attention out (phase q)
    d2 = dw2[0:96, :]                                    # simam d2 (phase simam)

    with TileContext(nc) as tc:
        with (
            tc.tile_pool(name="xio", bufs=2) as xio,
            tc.tile_pool(name="work", bufs=2) as work,
            tc.tile_pool(name="work1", bufs=2) as work1,
            tc.tile_pool(name="pout", bufs=2) as pout,
            tc.tile_pool(name="ppsum", bufs=3, space="PSUM") as ppsum,
        ):
            # ================= init =================
            nc.sync.dma_start(out=w1a_s[:], in_=w1a[:])
            nc.sync.dma_start(out=w1b_s[:], in_=w1b[:])
            nc.sync.dma_start(out=wdiag_s[:], in_=wdiag[:])
            nc.sync.dma_start(out=wtap_s[:], in_=wtap[:])
            nc.sync.dma_start(out=ident[:], in_=pat[:, 0:128])
            nc.sync.dma_start(out=ones_q[:], in_=pat[0:96, 128:256])
            nc.sync.dma_start(out=dv_ones[:], in_=pat[:, 256:384])
            nc.sync.dma_start(out=tmp_rep[:], in_=tmp_rep_d[:])
            nc.gpsimd.memset(ag_full[:], 0.0)
            nc.gpsimd.memset(av_l0[:], 0.0)
            nc.gpsimd.memset(av_l1[:], 0.0)
            nc.gpsimd.memset(vts[:], 0.0)
            for _b in range(4):
                nc.gpsimd.memset(vts[:, _b * 128 + 96:_b * 128 + 97], 1.0)
            nc.gpsimd.memset(half_s[:], 0.5)
            # pre borders (rows 0 and 129, cols 0 and 129)
            nc.gpsimd.memset(pre3[:, 0, :], 0.0)
            nc.gpsimd.memset(pre3[:, 129, :], 0.0)
            nc.gpsimd.memset(pre3[:, :, 0], 0.0)
            nc.gpsimd.memset(pre3[:, :, 129], 0.0)

            # ---- helpers ----
            xtiles = {}

            def conv_blk(s, j):
                wa = w1a_s[:, s * 128:(s + 1) * 128]
                wb = w1b_s[:, s * 128:(s + 1) * 128]
                if j % 2 == 0:
                    x0 = xio.tile([96, 2048], F16, tag="x0")
                    x1 = xio.tile([96, 2048], F16, tag="x1")
                    nc.sync.dma_start(out=x0[:],
                                      in_=xin[0:96, j * 1024:(j + 2) * 1024])
                    nc.sync.dma_start(out=x1[:],
                                      in_=xin[96:192, j * 1024:(j + 2) * 1024])
                    xtiles[0], xtiles[1] = x0, x1
                xo = (j % 2) * 1024
                x0, x1 = xtiles[0], xtiles[1]
                ps = ppsum.tile([128, 1024], F32, tag="big")
                for q in range(2):
                    sl = slice(q * 512, (q + 1) * 512)
                    xsl = slice(xo + q * 512, xo + (q + 1) * 512)
                    nc.tensor.matmul(ps[:, sl], wa, x0[:, xsl],
                                     start=True, stop=False)
                for q in range(2):
                    sl = slice(q * 512, (q + 1) * 512)
                    xsl = slice(xo + q * 512, xo + (q + 1) * 512)
                    nc.tensor.matmul(ps[:, sl], wb, x1[:, xsl],
                                     start=False, stop=True)
                nc.scalar.copy(pre3[:, 1 + 8 * j: 9 + 8 * j, 1:129], ps[:])

            def dw_blk(s, j):
                dst = dws[s][:, j * 1024:(j + 1) * 1024]
                pe_t = PE_TAPS[s]
                pd = ppsum.tile([128, 1024], F32, tag="big")
                for ti, (dy, dx) in enumerate(pe_t):
                    sl_d = WDIAG_SLOT[(s, dy, dx)]
                    dg = wdiag_s[:, sl_d * 128:(sl_d + 1) * 128]
                    for q in range(2):
                        rv = pre3[:, 1 + dy + 8 * j + 4 * q: 5 + dy + 8 * j + 4 * q,
                                  1 + dx: 129 + dx]
                        nc.tensor.matmul(pd[:, q * 512:(q + 1) * 512], dg, rv,
                                         start=(ti == 0), stop=(ti == len(pe_t) - 1))
                first = True
                for (dy, dx) in DVE_TAPS[s]:
                    ti = s * 9 + TAPS.index((dy, dx))
                    w_sc = wtap_s[:, ti:ti + 1]
                    rv = pre3[:, 1 + dy + 8 * j: 9 + dy + 8 * j, 1 + dx: 129 + dx]
                    nc.vector.scalar_tensor_tensor(
                        out=dst, in0=rv, scalar=w_sc, in1=(pd[:] if first else dst),
                        op0=OP.mult, op1=OP.add)
                    first = False
                for (dy, dx) in GP_TAPS[s]:
                    ti = s * 9 + TAPS.index((dy, dx))
                    w_sc = wtap_s[:, ti:ti + 1]
                    rv = pre3[:, 1 + dy + 8 * j: 9 + dy + 8 * j, 1 + dx: 129 + dx]
                    nc.gpsimd.scalar_tensor_tensor(
                        out=dst, in0=rv, scalar=w_sc, in1=dst,
                        op0=OP.mult, op1=OP.add)
                if first:
                    nc.scalar.copy(dst, pd[:])

            def pool_tree_ops(s):
                """Returns list of closures: f16 TT adder tree for strip s."""
                src = dws[s][96:128, :]
                stg = dws[STAGE_OF[s]][96:128, :]
                ops = []

                def lvl(dst_off, dst_xi, src_ap0, src_ap1):
                    def f():
                        d3 = stg[:, dst_off:dst_off + 1024 * dst_xi].rearrange(
                            "p (g xi) -> p g xi", xi=dst_xi)
                        nc.vector.tensor_tensor(out=d3, in0=src_ap0, in1=src_ap1,
                                                op=OP.add)
                    return f

                a3 = src.rearrange("p (g xi) -> p g xi", xi=16)
                ops.append(lvl(ST_L1, 8, a3[:, :, 0:8], a3[:, :, 8:16]))
                l1 = stg[:, ST_L1:ST_L1 + 8192].rearrange("p (g xi) -> p g xi", xi=8)
                ops.append(lvl(ST_L2, 4, l1[:, :, 0:4], l1[:, :, 4:8]))
                l2_ = stg[:, ST_L2:ST_L2 + 4096].rearrange("p (g xi) -> p g xi", xi=4)
                ops.append(lvl(ST_L3, 2, l2_[:, :, 0:2], l2_[:, :, 2:4]))
                l3 = stg[:, ST_L3:ST_L3 + 2048].rearrange("p (g xi) -> p g xi", xi=2)
                ops.append(lvl(ST_L4, 1, l3[:, :, 0:1], l3[:, :, 1:2]))
                # y-direction: r4 layout (by, yi, bx) with yi=16, bx=8
                r4 = stg[:, ST_L4:ST_L4 + 1024].rearrange(
                    "p (by yi bx) -> p by yi bx", by=8, bx=8)

                def ylvl(dst_off, dst_yi, src_ap0, src_ap1):
                    def f():
                        d3 = stg[:, dst_off:dst_off + 64 * dst_yi].rearrange(
                            "p (by yi bx) -> p by yi bx", by=8, bx=8)
                        nc.vector.tensor_tensor(out=d3, in0=src_ap0, in1=src_ap1,
                                                op=OP.add)
                    return f

                ops.append(ylvl(ST_M1, 8, r4[:, :, 0:8, :], r4[:, :, 8:16, :]))
                m1 = stg[:, ST_M1:ST_M1 + 512].rearrange(
                    "p (by yi bx) -> p by yi bx", by=8, bx=8)
                ops.append(ylvl(ST_M2, 4, m1[:, :, 0:4, :], m1[:, :, 4:8, :]))
                m2 = stg[:, ST_M2:ST_M2 + 256].rearrange(
                    "p (by yi bx) -> p by yi bx", by=8, bx=8)
                ops.append(ylvl(ST_M3, 2, m2[:, :, 0:2, :], m2[:, :, 2:4, :]))
                m3 = stg[:, ST_M3:ST_M3 + 128].rearrange(
                    "p (by yi bx) -> p by yi bx", by=8, bx=8)
                ops.append(ylvl(ST_M4, 1, m3[:, :, 0:1, :], m3[:, :, 1:2, :]))
                # extract: asum -> agf rows
                def extract():
                    nc.sync.dma_start(out=agf[s * 32:(s + 1) * 32, :],
                                      in_=stg[:, ST_M4:ST_M4 + 64])
                ops.append(extract)
                return ops

            def sweep(s, extras=()):
                extras = list(extras)
                conv_blk(s, 0)
                for j in range(1, NB2):
                    conv_blk(s, j)
                    dw_blk(s, j - 1)
                    for _ in range(2):
                        if extras:
                            extras.pop(0)()
                dw_blk(s, NB2 - 1)
                for e in extras:
                    e()

            def qnorm_blk(j):
                def f():
                    blk = slice(j * BLK2, (j + 1) * BLK2)
                    sq_q = work.tile([96, BLK2], F16, tag="sq_q")
                    nc.scalar.activation(sq_q[:], dw0[0:96, blk], AF.Square)
                    pq = ppsum.tile([128, BLK2], F32, tag="big")
                    for q in range(2):
                        sl = slice(q * 512, (q + 1) * 512)
                        nc.tensor.matmul(pq[:, sl], ones_q[:], sq_q[:, sl],
                                         start=True, stop=True)
                    rq = work1.tile([96, BLK2], F32, tag="rq", bufs=1)
                    nc.vector.reciprocal_approx_fast(out=rq[:], in_=pq[0:96, :])
                    rinv_q = work1.tile([96, BLK2], F16, tag="rinv_q")
                    nc.scalar.activation(rinv_q[:], rq[:], AF.Sqrt)
                    nc.gpsimd.tensor_tensor(out=dw0[0:96, blk],
                                            in0=dw0[0:96, blk],
                                            in1=rinv_q[:], op=OP.mult)
                return f

            def knorm_blk(j):
                def f():
                    blk = slice(j * BLK2, (j + 1) * BLK2)
                    sq_k = work.tile([96, BLK2], F16, tag="sq_k")
                    nc.scalar.activation(sq_k[:], dw1[0:96, blk], AF.Square)
                    pk = ppsum.tile([128, BLK2], F32, tag="big")
                    for q in range(2):
                        sl = slice(q * 512, (q + 1) * 512)
                        nc.tensor.matmul(pk[:, sl], ones_q[:], sq_k[:, sl],
                                         start=True, stop=True)
                    rk = work1.tile([96, BLK2], F32, tag="rk", bufs=1)
                    nc.vector.reciprocal_approx_fast(out=rk[:], in_=pk[0:96, :])
                    rinv_k = work1.tile([96, BLK2], F16, tag="rinv_k")
                    nc.scalar.activation(rinv_k[:], rk[:], AF.Sqrt)
                    nc.gpsimd.tensor_tensor(out=dw1[0:96, blk],
                                            in0=dw1[0:96, blk],
                                            in1=rinv_k[:], op=OP.mult)
                return f

            def interleave(a, b):
                out, a, b = [], list(a), list(b)
                while a or b:
                    if a:
                        out.append(a.pop(0))
                    if b:
                        out.append(b.pop(0))
                return out

            # ================= sweeps =================
            sweep(0)
            sweep(1, interleave(pool_tree_ops(0),
                                [qnorm_blk(j) for j in range(NB2)]))
            sweep(2, interleave(pool_tree_ops(1),
                                [knorm_blk(j) for j in range(NB2)]))

            if debug:
                nc.sync.dma_start(out=dbg_pre[:], in_=scratch[:])
                nc.sync.dma_start(out=dbg_dw0[:], in_=dw0[:])
                nc.sync.dma_start(out=dbg_dw1[:], in_=dw1[:])
                nc.sync.dma_start(out=dbg_dw2[:], in_=dw2[:])

            # ===== pool strip2 + agent assembly =======================
            for e in pool_tree_ops(2):
                e()
            # agfs = agf * temp/256; place head blocks into ag_full
            nc.vector.tensor_scalar(out=agfs[:], in0=agf[:],
                                    scalar1=tmp_rep[:], scalar2=1.0 / 256.0,
                                    op0=OP.mult, op1=OP.mult)
            for h in range(4):
                nc.sync.dma_start(
                    out=ag_full[h * 24:(h + 1) * 24, h * 64:(h + 1) * 64],
                    in_=agfs[h * 24:(h + 1) * 24, :])

            if debug:
                nc.sync.dma_start(out=dbg_ag[:], in_=ag_full[:])

            # ===== merged l2norm(q,k) + k-side ========================
            # per block j: normalize q/k cols, then k-side for chunks 8j..8j+8
            agv = ppsum.tile([128, 256], F32, tag="pin", bufs=1)

            def kside_grp(g):
                """4-chunk group: chunks 4g..4g+4 (pixels 512g..512(g+1))."""
                l2p = ppsum.tile([128, 1024], F32, tag="big")
                for ci in range(4):
                    c = 4 * g + ci
                    ssl = slice(c * 128, (c + 1) * 128)
                    nc.tensor.matmul(l2p[:, ci * 256:(ci + 1) * 256],
                                     dw1[0:96, ssl], ag_full[:],
                                     start=True, stop=True)
                e2t = work.tile([128, 1024], F16, tag="e2t")
                nc.scalar.activation(e2t[:], l2p[:], AF.Exp)
                for ci in range(4):
                    c = 4 * g + ci
                    ssl = slice(c * 128, (c + 1) * 128)
                    buf = c % 4
                    vtp = ppsum.tile([128, 96], F16, tag="big")
                    nc.tensor.transpose(vtp[:], dw2[0:96, ssl],
                                        ident[0:96, 0:96])
                    nc.vector.tensor_copy(vts[:, buf * 128:buf * 128 + 96],
                                          vtp[:])
                    nc.tensor.matmul(agv[:], vts[:, buf * 128:(buf + 1) * 128],
                                     e2t[:, ci * 256:(ci + 1) * 256],
                                     start=(c == 0), stop=(c == 4 * (NGRP // 2) - 1))

            for g in range(NGRP // 2):
                kside_grp(g)

            # ---- agv finalize: av_l = (agv / den)^T per head-pair ----
            nc.vector.tensor_copy(agv_sb[:], agv[0:97, :])
            if debug:
                nc.sync.dma_start(out=dbg_agv[:], in_=agv_sb[:])
            for hp in range(2):
                av_l = av_l0 if hp == 0 else av_l1
                avt = ppsum.tile([128, 97], F16, tag="big")
                nc.tensor.transpose(avt[:], agv_sb[:, hp * 128:(hp + 1) * 128],
                                    ident[0:97, 0:97])
                den32 = work1.tile([128, 1], F32, tag="den32")
                nc.vector.tensor_copy(den32[:], avt[:, 96:97])
                nc.vector.reciprocal_approx_fast(out=rden[:, hp:hp + 1],
                                                 in_=den32[:])
                h0, h1 = 2 * hp, 2 * hp + 1
                nc.vector.tensor_scalar(
                    out=av_l[0:64, 0:24], in0=avt[0:64, h0 * 24:h0 * 24 + 24],
                    scalar1=rden[0:64, hp:hp + 1], scalar2=None, op0=OP.mult)
                for w0 in (64, 96):
                    nc.vector.tensor_scalar(
                        out=av_l[w0:w0 + 32, 24:48],
                        in0=avt[w0:w0 + 32, h1 * 24:h1 * 24 + 24],
                        scalar1=rden[w0:w0 + 32, hp:hp + 1], scalar2=None,
                        op0=OP.mult)
            if debug:
                nc.sync.dma_start(out=dbg_qn[:], in_=dw0[:])
                nc.sync.dma_start(out=dbg_kn[:], in_=dw1[:])
                nc.sync.dma_start(out=dbg_av0[:], in_=av_l0[:, 0:48])
                nc.sync.dma_start(out=dbg_av1[:], in_=av_l1[:, 0:48])

            # ================= q-side + division ======================
            for j in range(NB2):
                for hp in range(2):
                    av_l = av_l0 if hp == 0 else av_l1
                    ag_cols = ag_full[:, hp * 128:(hp + 1) * 128]
                    blk = slice(j * BLK2, (j + 1) * BLK2)
                    l1p = ppsum.tile([128, BLK2], F32, tag="big")
                    for q in range(2):
                        sl = slice(j * BLK2 + q * 512, j * BLK2 + (q + 1) * 512)
                        psl = slice(q * 512, (q + 1) * 512)
                        nc.tensor.matmul(l1p[:, psl], ag_cols, dw0[0:96, sl],
                                         start=True, stop=True)
                    e1 = work.tile([128, BLK2], F16, tag="e1")
                    nc.scalar.activation(e1[:], l1p[:], AF.Exp)
                    op_ = ppsum.tile([128, BLK2], F32, tag="big")
                    od_ = ppsum.tile([128, BLK2], F32, tag="big")
                    for q in range(2):
                        psl = slice(q * 512, (q + 1) * 512)
                        nc.tensor.matmul(op_[:, psl], av_l[:], e1[:, psl],
                                         start=True, stop=True)
                    for q in range(2):
                        psl = slice(q * 512, (q + 1) * 512)
                        nc.tensor.matmul(od_[:, psl], dv_ones[:], e1[:, psl],
                                         start=True, stop=True)
                    rqs = work1.tile([48, BLK2], F32, tag="rqs")
                    nc.vector.reciprocal_approx_fast(out=rqs[:], in_=od_[0:48, :])
                    x2dump = work1.tile([48, BLK2], F16, tag="x2dump", bufs=1)
                    if hp == 0:
                        nc.vector.scalar_tensor_tensor(
                            out=x_attn[0:48, blk], in0=op_[0:48, :], scalar=0.0,
                            in1=rqs[:], op0=OP.bypass, op1=OP.mult,
                            accum_out=mu_parts[:, j:j + 1])
                        nc.scalar.activation(
                            x2dump[:], x_attn[0:48, blk], AF.Square,
                            accum_out=x2_parts[:, j:j + 1])
                    else:
                        xt = work1.tile([48, BLK2], F16, tag="xt")
                        nc.vector.scalar_tensor_tensor(
                            out=xt[:], in0=op_[0:48, :], scalar=0.0,
                            in1=rqs[:], op0=OP.bypass, op1=OP.mult,
                            accum_out=mu_parts[:, NB2 + j:NB2 + j + 1])
                        nc.scalar.activation(
                            x2dump[:], xt[:], AF.Square,
                            accum_out=x2_parts[:, NB2 + j:NB2 + j + 1])
                        nc.sync.dma_start(out=dw1[48:96, blk], in_=xt[:])

            if debug:
                nc.sync.dma_start(out=dbg_xa[:], in_=x_attn[:])
            # ================= SimAM =====================================
            # mu and sum(d2) via sum(x) and sum(x^2) so the tail needs no
            # separate accumulation pass.
            nc.vector.reduce_sum(mub[:, 0:1], mu_parts[:, 0:NB2], axis=AX.X)
            nc.vector.reduce_sum(mub[:, 1:2], mu_parts[:, NB2:2 * NB2], axis=AX.X)
            nc.vector.tensor_scalar(out=mub[:], in0=mub[:],
                                    scalar1=-1.0 / N, scalar2=None, op0=OP.mult)
            nc.sync.dma_start(out=mu_neg[0:48, :], in_=mub[:, 0:1])
            nc.sync.dma_start(out=mu_neg[48:96, :], in_=mub[:, 1:2])
            nc.vector.reduce_sum(x2b[:, 0:1], x2_parts[:, 0:NB2], axis=AX.X)
            nc.vector.reduce_sum(x2b[:, 1:2], x2_parts[:, NB2:2 * NB2], axis=AX.X)
            nc.sync.dma_start(out=x2s[0:48, :], in_=x2b[:, 0:1])
            nc.sync.dma_start(out=x2s[48:96, :], in_=x2b[:, 1:2])
            # sden = sum(x^2) - N*mu^2
            nc.vector.tensor_tensor(out=mu2[:], in0=mu_neg[:], in1=mu_neg[:],
                                    op=OP.mult)
            nc.vector.tensor_scalar(out=mu2[:], in0=mu2[:],
                                    scalar1=-float(N), scalar2=None, op0=OP.mult)
            nc.vector.tensor_tensor(out=sden[:], in0=x2s[:], in1=mu2[:],
                                    op=OP.add)
            nc.vector.tensor_scalar(out=sden[:], in0=sden[:],
                                    scalar1=4.0 / (N - 1), scalar2=4.0 * E_LAMBDA,
                                    op0=OP.mult, op1=OP.add)
            nc.vector.reciprocal_approx_fast(out=s_ch[:], in_=sden[:])
            # rsq = sqrt(s); bias_sg = -mu*rsq : t = (rsq*x + bias_sg)^2
            nc.scalar.activation(rsq[:], s_ch[:], AF.Sqrt)
            nc.vector.tensor_tensor(out=bsg[:], in0=mu_neg[:], in1=rsq[:],
                                    op=OP.mult)
            for j in range(NB2):
                blk = slice(j * BLK2, (j + 1) * BLK2)
                tt_ = pout.tile([96, BLK2], F16, tag="tt_", bufs=1)
                nc.vector.tensor_scalar(out=tt_[:], in0=x_attn[:, blk],
                                        scalar1=rsq[:], scalar2=bsg[:],
                                        op0=OP.mult, op1=OP.add)
                nc.vector.tensor_tensor(out=d2[:, blk], in0=tt_[:], in1=tt_[:],
                                        op=OP.mult)
            for j in range(NB2):
                blk = slice(j * BLK2, (j + 1) * BLK2)
                nc.scalar.activation(sig[0:96, blk], d2[:, blk], AF.Sigmoid,
                                     bias=half_s[:], scale=1.0)
                ob = pout.tile([96, BLK2], F16, tag="ob")
                if j % 2 == 0:
                    nc.vector.tensor_tensor(out=ob[:], in0=x_attn[:, blk],
                                            in1=sig[0:96, blk], op=OP.mult)
                else:
                    nc.gpsimd.tensor_tensor(out=ob[:], in0=x_attn[:, blk],
                                            in1=sig[0:96, blk], op=OP.mult)
                nc.sync.dma_start(out=out_d[:, blk], in_=ob[:])

    nc.compile()
    return nc


_NC = None
_NC_DEBUG = None


def _get_nc(debug=False):
    global _NC, _NC_DEBUG
    if debug:
        if _NC_DEBUG is None:
            _install_ntff_hook()
            _NC_DEBUG = build_nc(debug=True)
        return _NC_DEBUG
    if _NC is None:
        _install_ntff_hook()
        _NC = build_nc()
    return _NC


def _perm_rows(hg):
    """Conv output channel permutation, slab-major. Returns 384 indices into
    the 768 rows of w_qkv/w_dw."""
    q = np.arange(hg * 96, hg * 96 + 96)
    k = 192 + np.arange(hg * 96, hg * 96 + 96)
    v = 384 + np.arange(hg * 96, hg * 96 + 96)
    a = 576 + np.arange(hg * 96, hg * 96 + 96)
    return np.concatenate([q, a[0:32], k, a[32:64], v, a[64:96]])


def make_core_inputs(x, w_qkv, w_dw, temperature):
    """Host-side shard prep. Returns list of 8 input dicts."""
    x = np.asarray(x)
    w_qkv = np.asarray(w_qkv)
    w_dw = np.asarray(w_dw)
    temperature = np.asarray(temperature).reshape(8)
    in_maps = []
    for core in range(8):
        b, hg = core // 2, core % 2
        rows = _perm_rows(hg)
        W1 = w_qkv[rows, :, 0, 0]                        # [384, 192]
        W1T = np.ascontiguousarray(W1.T).astype(np.float16)
        wd9 = w_dw[rows, 0].reshape(384, 9).astype(np.float32)
        wdiag_h = np.zeros((128, NDIAG * 128), np.float16)
        wtap_h = np.zeros((128, 27), np.float32)
        for s in range(3):
            for t in range(9):
                wtap_h[:, s * 9 + t] = wd9[s * 128:(s + 1) * 128, t]
        for (s, dy, dx), idx in WDIAG_SLOT.items():
            t = (dy + 1) * 3 + (dx + 1)
            wdiag_h[np.arange(128), idx * 128 + np.arange(128)] = \
                wd9[s * 128:(s + 1) * 128, t].astype(np.float16)
        pat_h = np.zeros((128, 384), np.float16)
        pat_h[:, 0:128] = np.eye(128, dtype=np.float16)
        for h in range(4):
            pat_h[h * 24:(h + 1) * 24, 128 + h * 24:128 + (h + 1) * 24] = 1
        pat_h[0:64, 256:280] = 1         # dv_ones: even head rows -> cols 0:24
        pat_h[64:128, 280:304] = 1       # odd head rows -> cols 24:48
        heads = np.arange(hg * 4, hg * 4 + 4)
        t4 = temperature[heads].astype(np.float32)
        in_maps.append({
            "xin": x[b].reshape(192, N).astype(np.float16),
            "w1a": W1T[0:96].copy(),
            "w1b": W1T[96:192].copy(),
            "wdiag": wdiag_h,
            "wtap": wtap_h,
            "tmp_rep": np.repeat(t4, 24).reshape(96, 1).copy(),
            "pat": pat_h,
        })
    return in_maps


def _assemble(results):
    full = np.empty((B, C, H, W), np.float32)
    for core in range(8):
        b, hg = core // 2, core % 2
        full[b, hg * 96:(hg + 1) * 96] = \
            results[core]["out"].astype(np.float32).reshape(96, H, W)
    return full


def kernel(x, w_qkv, w_dw, temperature):
    nc = _get_nc()
    in_maps = make_core_inputs(x, w_qkv, w_dw, temperature)
    res = run_bass_kernel_spmd(nc, in_maps, list(range(8)))
    return _assemble(res.results)


def kernel_profiled(x, w_qkv, w_dw, temperature):
    nc = _get_nc()
    in_maps = make_core_inputs(x, w_qkv, w_dw, temperature)
    res = run_bass_kernel_spmd(nc, in_maps, list(range(8)), trace=True)
    return _assemble(res.results), res.exec_time_ns


def kernel_debug(x, w_qkv, w_dw, temperature):
    nc = _get_nc(debug=True)
    in_maps = make_core_inputs(x, w_qkv, w_dw, temperature)
    res = run_bass_kernel_spmd(nc, in_maps, list(range(8)))
    return res.results
